# revision 11
# baseline (speedup 1.0000x reference)
"""Trainium2 Bass kernel for nn_Attention (B=4, S=1024, D=1024, H=16).

Sharding: 8 cores = 4 batches x 2 query-halves. Core i handles batch i//2,
query rows [(i%2)*512, (i%2)*512+512). Each core computes the full K/V
projections for its batch (duplicated across the 2 cores sharing a batch),
all 16 heads of attention for its query slice, and the output projection.
No collectives; the output is gathered host-side.

Device dataflow (per core) — fp16 matmul operands, fp32 PSUM accumulation:
  - host passes pre-transposed qT [D,SQ], kT/vT [D,SK], and W*.T [D,D]
    (PE contracts over the partition dim, so both matmul operands need the
    contraction dim on partitions; transposing on host costs nothing on HW)
  - qhT[o,sq] = (Wq.T*SCALE).T-tiles @ qT     (o on partitions)
  - khT[o,sk] likewise; vh[sk, h, dh] natural via vT-as-stationary
  - scoresT[sk,sq] per head = khT-tile.T @ qhT; the two heads of a pair run
    as K=64 matmuls packed at PE row strips 0:64 / 64:128, writing the two
    halves of one [128, 2*SQ] PSUM tile -> ONE fused exp per pair
  - expT = exp(scoresT) on ACT (no max subtraction: |scores| < ~4 here,
    and softmax(x) == softmax(x - max) exactly)
  - ctxT_aug[dh+1, sq] += [vh | 1].T @ expT  (ones column makes row 64 the
    softmax denominator, riding free on the ctx matmul)
  - ctxT = ctxT_aug[0:64] * (1/ctxT_aug[64]) broadcast across partitions
  - out[sq,o] = ctxT-tiles.T @ Wo.T + bo      (natural layout -> direct DMA)

The attention phase is ACT(exp)-bound, so projection-matmul "filler" groups
are interleaved into the attention emission order to keep the PE busy (and
HAM-warm) while ACT chews through the exps.

Bias handling (exact): bq via per-partition add on the qh copy; bk dropped
(softmax is invariant to per-query score shifts); bv added after normalize
(softmax rows sum to 1); bo via a partition-broadcast tile.
"""

import sys

import numpy as np

if "/opt/trn_rl_repo" not in sys.path:
    sys.path.insert(0, "/opt/trn_rl_repo")

B, S, D, H = 4, 1024, 1024, 16
HD = D // H                      # 64
SCALE = 1.0 / float(np.sqrt(HD))
N_CORES = 8
SQ = S // 2                      # 512 query rows per core
SK = S                           # full key length
P = 128
NT = D // P                      # 8 feature tiles
SKT = SK // P                    # 8 key tiles
NPAIR = H // 2                   # 8 head pairs
NC2 = 512                        # max matmul free dim (one PSUM bank)

_CACHE = {}


def _build_program():
    from contextlib import ExitStack

    import concourse.bass as bass
    import concourse.tile as tile
    from concourse import bacc, mybir

    F32 = mybir.dt.float32
    F16 = mybir.dt.float16
    AF = mybir.ActivationFunctionType

    nc = bacc.Bacc(
        "TRN2", target_bir_lowering=False, debug=False, num_devices=N_CORES
    )

    qT_d = nc.dram_tensor("qT", [D, SQ], F16, kind="ExternalInput").ap()
    kT_d = nc.dram_tensor("kT", [D, SK], F16, kind="ExternalInput").ap()
    vT_d = nc.dram_tensor("vT", [D, SK], F16, kind="ExternalInput").ap()
    wqT_d = nc.dram_tensor("wqT", [D, D], F16, kind="ExternalInput").ap()
    wkT_d = nc.dram_tensor("wkT", [D, D], F16, kind="ExternalInput").ap()
    wvT_d = nc.dram_tensor("wvT", [D, D], F16, kind="ExternalInput").ap()
    woT_d = nc.dram_tensor("woT", [D, D], F16, kind="ExternalInput").ap()
    bq_d = nc.dram_tensor("bq", [D], F32, kind="ExternalInput").ap()
    bv_d = nc.dram_tensor("bv", [D], F32, kind="ExternalInput").ap()
    bo_d = nc.dram_tensor("bo", [D], F32, kind="ExternalInput").ap()
    out_d = nc.dram_tensor("out", [SQ, D], F32, kind="ExternalOutput").ap()

    mm = lambda *a, **k: nc.tensor.matmul(*a, **k)

    with tile.TileContext(nc) as tc, ExitStack() as ctx:
        persist = ctx.enter_context(tc.tile_pool(name="persist", bufs=1))
        epool = ctx.enter_context(tc.tile_pool(name="epool", bufs=3))
        rpool = ctx.enter_context(tc.tile_pool(name="rp", bufs=2))
        opool = ctx.enter_context(tc.tile_pool(name="outp", bufs=2))
        pp = ctx.enter_context(tc.tile_pool(name="pp", space="PSUM", bufs=2))
        pS = ctx.enter_context(tc.tile_pool(name="pS", space="PSUM", bufs=2))
        pX = ctx.enter_context(tc.tile_pool(name="pX", space="PSUM", bufs=1))

        # persistent data tiles (everything fits: ~140KB/partition)
        qT_sb = persist.tile([P, NT, SQ], F16)
        kT_sb = persist.tile([P, NT, SK], F16)
        vT_sb = persist.tile([P, NT, SK], F16)
        wq = persist.tile([P, NT, D], F16)
        wk = persist.tile([P, NT, D], F16)
        wv = persist.tile([P, NT, D], F16)
        wo = persist.tile([P, NT, D], F16)
        qhT = persist.tile([P, NT, SQ], F16)        # [o'%128, o'//128, sq]
        khT = persist.tile([P, NT, SK], F16)
        vh = persist.tile([P, SKT, H, HD + 1], F16)  # [sk%128, sk//128, h, .]
        ctxT = persist.tile([P, NT, SQ], F16)
        ctxU = persist.tile([P, NT, SQ], F16)    # unnormalized ctx (PSUM drain)
        bq_sb = persist.tile([P, NT], F32)
        bv_sb = persist.tile([P, NT], F32)
        bo_bc = persist.tile([P, D], F32)

        # input DMAs, in consumption order (vT/wv -> kT/wk -> qT/wq -> wo);
        # first halves of vT/wv land first so a_group(j<4, c=0) starts early
        for h2 in range(2):
            for kk in range(NT):
                nc.sync.dma_start(vT_sb[:, kk, h2 * NC2:(h2 + 1) * NC2],
                                  vT_d[kk * P:(kk + 1) * P, h2 * NC2:(h2 + 1) * NC2])
                nc.sync.dma_start(wv[:, kk, h2 * NC2:(h2 + 1) * NC2],
                                  wvT_d[kk * P:(kk + 1) * P, h2 * NC2:(h2 + 1) * NC2])
        for kk in range(NT):
            nc.sync.dma_start(kT_sb[:, kk, :], kT_d[kk * P:(kk + 1) * P, :])
            nc.sync.dma_start(wk[:, kk, :], wkT_d[kk * P:(kk + 1) * P, :])
        for kk in range(NT):
            nc.sync.dma_start(qT_sb[:, kk, :], qT_d[kk * P:(kk + 1) * P, :])
            nc.sync.dma_start(wq[:, kk, :], wqT_d[kk * P:(kk + 1) * P, :])
        for kk in range(NT):
            nc.sync.dma_start(wo[:, kk, :], woT_d[kk * P:(kk + 1) * P, :])
        nc.gpsimd.dma_start(out=bq_sb, in_=bq_d.rearrange("(m p) -> p m", p=P))
        nc.gpsimd.dma_start(out=bv_sb, in_=bv_d.rearrange("(m p) -> p m", p=P))
        bo_bcast_ap = bass.AP(
            tensor=bo_d.tensor, offset=bo_d.offset, ap=[[0, P]] + list(bo_d.ap)
        )
        nc.gpsimd.dma_start(out=bo_bc, in_=bo_bcast_ap)
        for j in range(SKT):
            nc.vector.memset(vh[:, j, :, HD].bitcast(mybir.dt.uint16), 0x3C00)

        # ---- emit-group helpers (each = one 8-MM PSUM accumulation) ----
        def a_group(j, c):  # v-proj: vh[:, j, heads c*8..c*8+7]
            psa = pp.tile([P, NC2], F32, name="ppt")
            for kk in range(NT):
                mm(psa, vT_sb[:, kk, j * P:(j + 1) * P],
                   wv[:, kk, c * NC2:(c + 1) * NC2],
                   start=kk == 0, stop=kk == NT - 1)
            nc.vector.tensor_copy(
                vh[:, j, c * 8:(c + 1) * 8, 0:HD],
                psa.rearrange("p (h d) -> p h d", d=HD),
            )

        def b_group(m, c):  # k-proj: khT[:, m, c*512:...]
            psb = pp.tile([P, NC2], F32, name="ppt")
            for kk in range(NT):
                mm(psb, wk[:, kk, m * P:(m + 1) * P],
                   kT_sb[:, kk, c * NC2:(c + 1) * NC2],
                   start=kk == 0, stop=kk == NT - 1)
            nc.vector.tensor_copy(khT[:, m, c * NC2:(c + 1) * NC2], psb)

        def c_group(m):  # q-proj: qhT[:, m, :]
            psc = pp.tile([P, NC2], F32, name="ppt")
            for kk in range(NT):
                mm(psc, wq[:, kk, m * P:(m + 1) * P], qT_sb[:, kk, :],
                   start=kk == 0, stop=kk == NT - 1)
            nc.vector.tensor_scalar_add(qhT[:, m, :], psc, bq_sb[:, m:m + 1])

        def e_group(sqt, c):  # out-proj: out rows sqt*128, cols c*512
            pse = pp.tile([P, NC2], F32, name="ppt")
            for kk in range(NT):
                mm(pse, ctxT[:, kk, sqt * P:(sqt + 1) * P],
                   wo[:, kk, c * NC2:(c + 1) * NC2],
                   start=kk == 0, stop=kk == NT - 1)
            o_sb = opool.tile([P, NC2], F32, name="o_sb")
            nc.vector.tensor_add(o_sb, pse, bo_bc[:, c * NC2:(c + 1) * NC2])
            nc.sync.dma_start(
                out_d[sqt * P:(sqt + 1) * P, c * NC2:(c + 1) * NC2], o_sb
            )

        # ---- filler stream: projection groups interleaved into attention ----
        filler = []          # list of (label, emit_fn)
        emitted = set()

        def drain_until(labels):
            for lbl, fn in filler:
                if lbl not in emitted:
                    emitted.add(lbl)
                    fn()
                if all(x in emitted for x in labels):
                    return

        def drain_next(n=1):
            done = 0
            for lbl, fn in filler:
                if lbl not in emitted:
                    emitted.add(lbl)
                    fn()
                    done += 1
                    if done >= n:
                        return

        # ---- attention for one head pair ----
        def scores(t, j):
            sp = pS.tile([P, 2, SQ], F32, name="sp")
            mm(sp[:, 0, :], khT[0:HD, t, j * P:(j + 1) * P], qhT[0:HD, t, :],
               start=True, stop=True)
            mm(sp[:, 1, :], khT[HD:P, t, j * P:(j + 1) * P], qhT[HD:P, t, :],
               start=True, stop=True)
            return sp

        def d_pair(t, pcx0, pcx1):
            sps = {0: scores(t, 0)}
            for j in range(SKT):
                ep = epool.tile([P, 2, SQ], F16, name="ep")
                nc.scalar.activation(ep, sps.pop(j), AF.Exp)
                if j + 1 < SKT:
                    sps[j + 1] = scores(t, j + 1)
                if j % 2 == 0:
                    drain_next(1)
                mm(pcx0, vh[:, j, 2 * t, :], ep[:, 0, :],
                   start=j == 0, stop=j == SKT - 1)
                mm(pcx1, vh[:, j, 2 * t + 1, :], ep[:, 1, :],
                   start=j == 0, stop=j == SKT - 1)
            # quick PSUM->SBUF drain so the pX slots free fast; the actual
            # normalization runs later, off the PE critical path
            nc.vector.tensor_copy(ctxU[0:HD, t, :], pcx0[0:HD, :])
            nc.vector.tensor_copy(ctxU[HD:P, t, :], pcx1[0:HD, :])
            r0 = rpool.tile([1, SQ], F32, name="r0")
            r1 = rpool.tile([1, SQ], F32, name="r1")
            nc.vector.reciprocal(r0, pcx0[HD:HD + 1, :])
            nc.vector.reciprocal(r1, pcx1[HD:HD + 1, :])
            return r0, r1

        def normalize(t, r0, r1):
            rb0 = rpool.tile([P, SQ], F32, name="rb0")
            rb1 = rpool.tile([P, SQ], F32, name="rb1")
            nc.gpsimd.partition_broadcast(rb0, r0)
            nc.gpsimd.partition_broadcast(rb1, r1)
            nc.vector.tensor_mul(ctxT[0:HD, t, :], ctxU[0:HD, t, :], rb0[0:HD, :])
            nc.vector.tensor_mul(ctxT[HD:P, t, :], ctxU[HD:P, t, :], rb1[HD:P, :])
            nc.vector.tensor_scalar_add(
                ctxT[:, t, :], ctxT[:, t, :], bv_sb[:, t:t + 1]
            )

        # ---- emission schedule ----
        # upfront: enough to start attention pair 0
        for j in range(SKT):
            a_group(j, 0)            # vh heads 0-7
        b_group(0, 0); b_group(0, 1)  # khT pair 0
        c_group(0)                    # qhT pair 0

        # order matters: pair t needs b{t}/c{t}; pairs 4-7 also need all a{j}
        # (vh heads 8-15), so the a-groups sit before b4..c7 in the stream.
        for m in range(1, 4):
            filler.append((f"b{m}a", lambda m=m: b_group(m, 0)))
            filler.append((f"b{m}b", lambda m=m: b_group(m, 1)))
            filler.append((f"c{m}", lambda m=m: c_group(m)))
        for j in range(SKT):
            filler.append((f"a{j}", lambda j=j: a_group(j, 1)))
        for m in range(4, NT):
            filler.append((f"b{m}a", lambda m=m: b_group(m, 0)))
            filler.append((f"b{m}b", lambda m=m: b_group(m, 1)))
            filler.append((f"c{m}", lambda m=m: c_group(m)))

        for t in range(NPAIR):
            need = []
            if t >= 1:
                need += [f"b{t}a", f"b{t}b", f"c{t}"]
            if t >= 4:
                need += [f"a{j}" for j in range(SKT)]
            if need:
                drain_until(need)
            pcx0 = pX.tile([HD + 1, SQ], F32, name="pcx0")
            pcx1 = pX.tile([HD + 1, SQ], F32, name="pcx1")
            r0, r1 = d_pair(t, pcx0, pcx1)
            normalize(t, r0, r1)

        drain_until([lbl for lbl, _ in filler])

        # ---- output projection ----
        for sqt in range(SQ // P):
            for c in range(2):
                e_group(sqt, c)

    nc.compile()
    return nc


def get_program():
    if "nc" not in _CACHE:
        _CACHE["nc"] = _build_program()
    return _CACHE["nc"]


def make_in_maps(q, k, v, Wq, bq, Wk, bk, Wv, bv, Wo, bo):
    f32 = lambda x: np.ascontiguousarray(np.asarray(x, dtype=np.float32))
    h = lambda x: np.ascontiguousarray(np.asarray(x, dtype=np.float16))
    q, k, v = np.asarray(q, np.float32), np.asarray(k, np.float32), \
        np.asarray(v, np.float32)
    wqT = h(np.asarray(Wq, np.float32).T * np.float32(SCALE))
    wkT = h(np.asarray(Wk, np.float32).T)
    wvT = h(np.asarray(Wv, np.float32).T)
    woT = h(np.asarray(Wo, np.float32).T)
    bqs = f32(bq) * np.float32(SCALE)
    bv_, bo_ = f32(bv), f32(bo)
    # bk is exactly irrelevant: it shifts every score in a query row equally.
    kTs = [h(k[b].T) for b in range(B)]
    vTs = [h(v[b].T) for b in range(B)]
    in_maps = []
    for core in range(N_CORES):
        b, half = divmod(core, 2)
        qT_c = h(q[b, half * SQ:(half + 1) * SQ, :].T)
        in_maps.append({
            "qT": qT_c, "kT": kTs[b], "vT": vTs[b],
            "wqT": wqT, "wkT": wkT, "wvT": wvT, "woT": woT,
            "bq": bqs, "bv": bv_, "bo": bo_,
        })
    return in_maps


def gather_out(results):
    out = np.empty((B, S, D), dtype=np.float32)
    for core in range(N_CORES):
        b, half = divmod(core, 2)
        out[b, half * SQ:(half + 1) * SQ, :] = results[core]["out"]
    return out


def kernel(q, k, v, Wq, bq, Wk, bk, Wv, bv, Wo, bo):
    from concourse.bass_utils import run_bass_kernel_spmd

    nc = get_program()
    in_maps = make_in_maps(q, k, v, Wq, bq, Wk, bk, Wv, bv, Wo, bo)
    res = run_bass_kernel_spmd(nc, in_maps, list(range(N_CORES)))
    return gather_out(res.results)


# revision 13
# speedup vs baseline: 1.2492x; 1.2492x over previous
"""Trainium2 Bass kernel for nn_Attention (B=4, S=1024, D=1024, H=16).

Sharding: 8 cores = 4 batches x 2 query-halves. Core i handles batch i//2,
query rows [(i%2)*512, (i%2)*512+512). Each core computes the full K/V
projections for its batch (duplicated across the 2 cores sharing a batch),
all 16 heads of attention for its query slice, and the output projection.
No collectives; the output is gathered host-side.

Device dataflow (per core) — fp16 matmul operands, fp32 PSUM accumulation:
  - host passes pre-transposed qT [D,SQ], kT/vT [D,SK], W{v,o}.T [D,D], and
    m-blocked W{k,q}.T [NT,D,128] (PE contracts over the partition dim, so
    both matmul operands need the contraction dim on partitions; transposing
    and blocking on host costs nothing on HW)
  - qhT[o,sq] = (Wq.T*SCALE).T-tiles @ qT     (o on partitions)
  - khT[o,sk] likewise; vh[sk, h, dh] natural via vT-as-stationary
  - scoresT[sk,sq] per head = khT-tile.T @ qhT; the two heads of a pair run
    as K=64 matmuls packed at PE row strips 0:64 / 64:128, writing the two
    halves of one [128, 2*SQ] PSUM tile -> ONE fused exp per pair
  - expT = exp(scoresT) on ACT (no max subtraction: |scores| < ~4 here,
    and softmax(x) == softmax(x - max) exactly)
  - ctxT_aug[dh+1, sq] += [vh | 1].T @ expT  (ones column makes row 64 the
    softmax denominator, riding free on the ctx matmul)
  - ctx PSUM is drained fast (ACT copies + approx-reciprocal of the sum
    row); the normalization multiply runs later, off the critical path
  - out[sq,o] = ctxT-tiles.T @ Wo.T + bo      (natural layout -> direct DMA)

The attention phase is ACT(exp)-bound per-step, so projection-matmul
"filler" groups are interleaved into the attention emission order (with
need-driven draining) to keep the PE busy and HAM-warm throughout.

Bias handling (exact): bq via per-partition add on the qh copy; bk dropped
(softmax is invariant to per-query score shifts); bv added after normalize
(softmax rows sum to 1); bo via a partition-broadcast tile.
"""

import sys

import numpy as np

if "/opt/trn_rl_repo" not in sys.path:
    sys.path.insert(0, "/opt/trn_rl_repo")

B, S, D, H = 4, 1024, 1024, 16
HD = D // H                      # 64
SCALE = 1.0 / float(np.sqrt(HD))
N_CORES = 8
SQ = S // 2                      # 512 query rows per core
SK = S                           # full key length
P = 128
NT = D // P                      # 8 feature tiles
SKT = SK // P                    # 8 key tiles
NPAIR = H // 2                   # 8 head pairs
NC2 = 512                        # max matmul free dim (one PSUM bank)

_CACHE = {}


def _build_program():
    from contextlib import ExitStack

    import concourse.bass as bass
    import concourse.tile as tile
    from concourse import bacc, mybir

    F32 = mybir.dt.float32
    F16 = mybir.dt.float16
    AF = mybir.ActivationFunctionType

    nc = bacc.Bacc(
        "TRN2", target_bir_lowering=False, debug=False, num_devices=N_CORES
    )

    qT_d = nc.dram_tensor("qT", [D, SQ], F16, kind="ExternalInput").ap()
    kT_d = nc.dram_tensor("kT", [D, SK], F16, kind="ExternalInput").ap()
    vT_d = nc.dram_tensor("vT", [D, SK], F16, kind="ExternalInput").ap()
    wqT_d = nc.dram_tensor("wqT", [NT, D, P], F16, kind="ExternalInput").ap()
    wkT_d = nc.dram_tensor("wkT", [NT, D, P], F16, kind="ExternalInput").ap()
    wvT_d = nc.dram_tensor("wvT", [D, D], F16, kind="ExternalInput").ap()
    woT_d = nc.dram_tensor("woT", [D, D], F16, kind="ExternalInput").ap()
    bq_d = nc.dram_tensor("bq", [D], F32, kind="ExternalInput").ap()
    bv_d = nc.dram_tensor("bv", [D], F32, kind="ExternalInput").ap()
    bo_d = nc.dram_tensor("bo", [D], F32, kind="ExternalInput").ap()
    out_d = nc.dram_tensor("out", [SQ, D], F32, kind="ExternalOutput").ap()

    mm = lambda *a, **k: nc.tensor.matmul(*a, **k)

    with tile.TileContext(nc) as tc, ExitStack() as ctx:
        persist = ctx.enter_context(tc.tile_pool(name="persist", bufs=1))
        epool = ctx.enter_context(tc.tile_pool(name="epool", bufs=3))
        rpool = ctx.enter_context(tc.tile_pool(name="rp", bufs=2))
        opool = ctx.enter_context(tc.tile_pool(name="outp", bufs=2))
        pp = ctx.enter_context(tc.tile_pool(name="pp", space="PSUM", bufs=2))
        pS = ctx.enter_context(tc.tile_pool(name="pS", space="PSUM", bufs=2))
        pX = ctx.enter_context(tc.tile_pool(name="pX", space="PSUM", bufs=1))

        # persistent data tiles
        qT_sb = persist.tile([P, NT, SQ], F16)
        kT_sb = persist.tile([P, NT, SK], F16)
        vT_sb = persist.tile([P, NT, SK], F16)
        wq = persist.tile([P, NT, D], F16)
        wk = persist.tile([P, NT, D], F16)
        wv = persist.tile([P, NT, D], F16)
        wo = persist.tile([P, NT, D], F16)
        qhT = persist.tile([P, NT, SQ], F16)        # [o'%128, o'//128, sq]
        khT = persist.tile([P, NT, SK], F16)
        vh = persist.tile([P, SKT, H, HD + 1], F16)  # [sk%128, sk//128, h, .]
        ctxT = persist.tile([P, NT, SQ], F16)
        ctxU = persist.tile([P, NT, SQ], F16)    # unnormalized ctx (drain)
        bq_sb = persist.tile([P, NT], F32)
        bv_sb = persist.tile([P, NT], F32)
        bo_bc = persist.tile([P, D], F32)

        # input DMAs ordered by first use: the attention pair-0 critical path
        # (vT, wv-c0, kT, wk-m0, qT, wq-m0) lands first
        for kk in range(NT):
            nc.sync.dma_start(vT_sb[:, kk, :], vT_d[kk * P:(kk + 1) * P, :])
        for kk in range(NT):
            nc.sync.dma_start(wv[:, kk, 0:NC2],
                              wvT_d[kk * P:(kk + 1) * P, 0:NC2])
        for kk in range(NT):
            nc.sync.dma_start(kT_sb[:, kk, :], kT_d[kk * P:(kk + 1) * P, :])

        def load_wblock(w_sb, w_blk_d, m):
            nc.sync.dma_start(
                w_sb[:, :, m * P:(m + 1) * P],
                w_blk_d[m].rearrange("(kk p) c -> p kk c", p=P),
            )

        load_wblock(wk, wkT_d, 0)
        for kk in range(NT):
            nc.sync.dma_start(qT_sb[:, kk, :], qT_d[kk * P:(kk + 1) * P, :])
        load_wblock(wq, wqT_d, 0)
        for m in range(1, 4):
            load_wblock(wk, wkT_d, m)
            load_wblock(wq, wqT_d, m)
        for kk in range(NT):
            nc.sync.dma_start(wv[:, kk, NC2:D],
                              wvT_d[kk * P:(kk + 1) * P, NC2:D])
        for m in range(4, NT):
            load_wblock(wk, wkT_d, m)
            load_wblock(wq, wqT_d, m)
        for kk in range(NT):
            nc.sync.dma_start(wo[:, kk, :], woT_d[kk * P:(kk + 1) * P, :])
        nc.gpsimd.dma_start(out=bq_sb, in_=bq_d.rearrange("(m p) -> p m", p=P))
        nc.gpsimd.dma_start(out=bv_sb, in_=bv_d.rearrange("(m p) -> p m", p=P))
        bo_bcast_ap = bass.AP(
            tensor=bo_d.tensor, offset=bo_d.offset, ap=[[0, P]] + list(bo_d.ap)
        )
        nc.gpsimd.dma_start(out=bo_bc, in_=bo_bcast_ap)
        for j in range(SKT):
            nc.vector.memset(vh[:, j, :, HD].bitcast(mybir.dt.uint16), 0x3C00)

        # ---- emit-group helpers (each = one 8-MM PSUM accumulation) ----
        def a_group(j, c):  # v-proj: vh[:, j, heads c*8..c*8+7]
            psa = pp.tile([P, NC2], F32, name="ppt")
            for kk in range(NT):
                mm(psa, vT_sb[:, kk, j * P:(j + 1) * P],
                   wv[:, kk, c * NC2:(c + 1) * NC2],
                   start=kk == 0, stop=kk == NT - 1)
            nc.vector.tensor_copy(
                vh[:, j, c * 8:(c + 1) * 8, 0:HD],
                psa.rearrange("p (h d) -> p h d", d=HD),
            )

        def b_group(m, c):  # k-proj: khT[:, m, c*512:...]
            psb = pp.tile([P, NC2], F32, name="ppt")
            for kk in range(NT):
                mm(psb, wk[:, kk, m * P:(m + 1) * P],
                   kT_sb[:, kk, c * NC2:(c + 1) * NC2],
                   start=kk == 0, stop=kk == NT - 1)
            nc.vector.tensor_copy(khT[:, m, c * NC2:(c + 1) * NC2], psb)

        def c_group(m):  # q-proj: qhT[:, m, :]
            psc = pp.tile([P, NC2], F32, name="ppt")
            for kk in range(NT):
                mm(psc, wq[:, kk, m * P:(m + 1) * P], qT_sb[:, kk, :],
                   start=kk == 0, stop=kk == NT - 1)
            nc.vector.tensor_scalar_add(qhT[:, m, :], psc, bq_sb[:, m:m + 1])

        def e_group(sqt, c):  # out-proj: out rows sqt*128, cols c*512
            pse = pp.tile([P, NC2], F32, name="ppt")
            for kk in range(NT):
                mm(pse, ctxT[:, kk, sqt * P:(sqt + 1) * P],
                   wo[:, kk, c * NC2:(c + 1) * NC2],
                   start=kk == 0, stop=kk == NT - 1)
            o_sb = opool.tile([P, NC2], F32, name="o_sb")
            nc.vector.tensor_add(o_sb, pse, bo_bc[:, c * NC2:(c + 1) * NC2])
            nc.sync.dma_start(
                out_d[sqt * P:(sqt + 1) * P, c * NC2:(c + 1) * NC2], o_sb
            )

        # ---- filler stream with need-driven drains ----
        filler = []          # ordered list of (label, emit_fn)
        emitted = set()

        def drain_until(labels):
            todo = [x for x in labels if x not in emitted]
            if not todo:
                return
            for lbl, fn in filler:
                if lbl not in emitted:
                    emitted.add(lbl)
                    fn()
                if all(x in emitted for x in todo):
                    return

        def drain_next(n=1):
            done = 0
            for lbl, fn in filler:
                if lbl not in emitted:
                    emitted.add(lbl)
                    fn()
                    done += 1
                    if done >= n:
                        return

        # ---- attention ----
        def scores(t, j):
            sp = pS.tile([P, 2, SQ], F32, name="sp")
            mm(sp[:, 0, :], khT[0:HD, t, j * P:(j + 1) * P], qhT[0:HD, t, :],
               start=True, stop=True)
            mm(sp[:, 1, :], khT[HD:P, t, j * P:(j + 1) * P], qhT[HD:P, t, :],
               start=True, stop=True)
            return sp

        def d_pair(t, pcx0, pcx1):
            c = t // 4
            sps = {0: scores(t, 0)}
            for j in range(SKT):
                ep = epool.tile([P, 2, SQ], F16, name="ep")
                nc.scalar.activation(ep, sps.pop(j), AF.Exp)
                if j + 1 < SKT:
                    sps[j + 1] = scores(t, j + 1)
                drain_until([f"a{j}c{c}"])
                drain_next(1)
                mm(pcx0, vh[:, j, 2 * t, :], ep[:, 0, :],
                   start=j == 0, stop=j == SKT - 1)
                mm(pcx1, vh[:, j, 2 * t + 1, :], ep[:, 1, :],
                   start=j == 0, stop=j == SKT - 1)
            # fast PSUM drain: ACT copies the ctx rows, DVE approximates the
            # reciprocal of the sum row; normalization happens later
            nc.scalar.copy(ctxU[0:HD, t, :], pcx0[0:HD, :])
            nc.scalar.copy(ctxU[HD:P, t, :], pcx1[0:HD, :])
            se0 = rpool.tile([1, SQ], F32, name="se0")
            se1 = rpool.tile([1, SQ], F32, name="se1")
            nc.vector.tensor_copy(se0, pcx0[HD:HD + 1, :])
            nc.vector.tensor_copy(se1, pcx1[HD:HD + 1, :])
            r0 = rpool.tile([1, SQ], F32, name="r0")
            r1 = rpool.tile([1, SQ], F32, name="r1")
            nc.vector.reciprocal_approx_fast(r0, se0)
            nc.vector.reciprocal_approx_fast(r1, se1)
            return r0, r1

        def normalize(t, r0, r1):
            rb0 = rpool.tile([P, SQ], F32, name="rb0")
            rb1 = rpool.tile([P, SQ], F32, name="rb1")
            nc.gpsimd.partition_broadcast(rb0, r0)
            nc.gpsimd.partition_broadcast(rb1, r1)
            nc.vector.tensor_mul(ctxT[0:HD, t, :], ctxU[0:HD, t, :],
                                 rb0[0:HD, :])
            nc.vector.tensor_mul(ctxT[HD:P, t, :], ctxU[HD:P, t, :],
                                 rb1[HD:P, :])
            nc.vector.tensor_scalar_add(
                ctxT[:, t, :], ctxT[:, t, :], bv_sb[:, t:t + 1]
            )

        # ---- emission schedule ----
        a_group(0, 0)
        a_group(1, 0)
        b_group(0, 0)
        b_group(0, 1)
        c_group(0)
        emitted.add("a0c0")
        emitted.add("a1c0")
        filler.append(("a0c0", lambda: None))
        filler.append(("a1c0", lambda: None))

        for j in range(2, SKT):
            filler.append((f"a{j}c0", lambda j=j: a_group(j, 0)))
        for m in range(1, 4):
            filler.append((f"b{m}a", lambda m=m: b_group(m, 0)))
            filler.append((f"b{m}b", lambda m=m: b_group(m, 1)))
            filler.append((f"c{m}", lambda m=m: c_group(m)))
        for j in range(SKT):
            filler.append((f"a{j}c1", lambda j=j: a_group(j, 1)))
        for m in range(4, NT):
            filler.append((f"b{m}a", lambda m=m: b_group(m, 0)))
            filler.append((f"b{m}b", lambda m=m: b_group(m, 1)))
            filler.append((f"c{m}", lambda m=m: c_group(m)))

        prev = None
        for t in range(NPAIR):
            if t >= 1:
                drain_until([f"b{t}a", f"b{t}b", f"c{t}"])
            pcx0 = pX.tile([HD + 1, SQ], F32, name="pcx0")
            pcx1 = pX.tile([HD + 1, SQ], F32, name="pcx1")
            r0, r1 = d_pair(t, pcx0, pcx1)
            if prev is not None:
                normalize(*prev)
            prev = (t, r0, r1)
        normalize(*prev)

        drain_until([lbl for lbl, _ in filler])

        # ---- output projection ----
        for sqt in range(SQ // P):
            for c in range(2):
                e_group(sqt, c)

    nc.compile()
    return nc


def get_program():
    if "nc" not in _CACHE:
        _CACHE["nc"] = _build_program()
    return _CACHE["nc"]


def make_in_maps(q, k, v, Wq, bq, Wk, bk, Wv, bv, Wo, bo):
    f32 = lambda x: np.ascontiguousarray(np.asarray(x, dtype=np.float32))
    h = lambda x: np.ascontiguousarray(np.asarray(x, dtype=np.float16))
    blk = lambda wT: np.ascontiguousarray(
        np.asarray(wT, np.float16).reshape(D, NT, P).transpose(1, 0, 2)
    )
    q, k, v = np.asarray(q, np.float32), np.asarray(k, np.float32), \
        np.asarray(v, np.float32)
    wqT = blk(np.asarray(Wq, np.float32).T * np.float32(SCALE))
    wkT = blk(np.asarray(Wk, np.float32).T)
    wvT = h(np.asarray(Wv, np.float32).T)
    woT = h(np.asarray(Wo, np.float32).T)
    bqs = f32(bq) * np.float32(SCALE)
    bv_, bo_ = f32(bv), f32(bo)
    # bk is exactly irrelevant: it shifts every score in a query row equally.
    kTs = [h(k[b].T) for b in range(B)]
    vTs = [h(v[b].T) for b in range(B)]
    in_maps = []
    for core in range(N_CORES):
        b, half = divmod(core, 2)
        qT_c = h(q[b, half * SQ:(half + 1) * SQ, :].T)
        in_maps.append({
            "qT": qT_c, "kT": kTs[b], "vT": vTs[b],
            "wqT": wqT, "wkT": wkT, "wvT": wvT, "woT": woT,
            "bq": bqs, "bv": bv_, "bo": bo_,
        })
    return in_maps


def gather_out(results):
    out = np.empty((B, S, D), dtype=np.float32)
    for core in range(N_CORES):
        b, half = divmod(core, 2)
        out[b, half * SQ:(half + 1) * SQ, :] = results[core]["out"]
    return out


def kernel(q, k, v, Wq, bq, Wk, bk, Wv, bv, Wo, bo):
    from concourse.bass_utils import run_bass_kernel_spmd

    nc = get_program()
    in_maps = make_in_maps(q, k, v, Wq, bq, Wk, bk, Wv, bv, Wo, bo)
    res = run_bass_kernel_spmd(nc, in_maps, list(range(N_CORES)))
    return gather_out(res.results)
